# revision 46
# baseline (speedup 1.0000x reference)
"""Trainium2 Bass kernel for a 12-head causal attention block (GPT-2 style).

Problem: x:[4,2048,768] -> qkv = x@W_attn+b_attn, causal softmax attention
(12 heads, d=64), out @ W_proj + b_proj.

Sharding over 8 NeuronCores: core c handles batch b=c//2 (data parallel) and
head-group hg=c%2 (6 heads, tensor parallel on the qkv columns / proj rows).
Each core returns a partial projection output; the host sums the two
head-group partials per batch and adds the output bias (b_proj plus the
b_v@W_proj term: softmax rows sum to 1, so the v-bias contributes a constant
vector to the attention output and is folded host-side).

Per-core dataflow (inputs bf16; matmul accumulation fp32):
  - xT [emb, seq] comes straight from DRAM via DMA-transpose (bf16).
  - qkT = W-tiles.T @ xT -> qT,kT per head-pair [128,2048] (even head rows
    0-63, odd head rows 64-127); v in natural [seq, d] layout interleaved
    with ones columns (ones give the softmax denominators for free in the
    P@V matmul's 65th output row).
  - scores S^T[k,q] per 128k x 512q block: the two heads of a pair run
    ROW-PACKED (tile_position (0,0)/(64,0)) and execute concurrently in the
    PE array; one ACT exp call covers both heads.  Upper-triangle blocks are
    skipped; diagonal-crossing triangles zeroed post-exp with one 3D-batched
    gpsimd affine_select.  No max-subtraction needed (|scores/8| small).
  - P@V accumulates [attn-out^T ; den] in PSUM over k-tiles (M=65).
  - normalization: DVE reciprocal_approx_fast on the den row (~18 bits,
    5x faster than exact), gpsimd partition_broadcast to 64 rows, DVE
    multiply -> attnT (bf16).  Odd head bounced to rows 64-127 via DMA.
  - proj: y[128q,768] accumulated over the 3 head-pair k-tiles in 384-col
    chunks.
  - PE saturation: the attention j-loop is ACT(exp)-bound, so the qkv
    projection matmuls for LATER pairs and the output projection are emitted
    as filler blocks interleaved between j-iterations, keeping the PE busy
    (and HAM-warm) throughout instead of running phases serially.
"""

import os
import ml_dtypes
import numpy as np

N_HEAD = 12
N_EMBD = 768
HEAD_DIM = 64
B, S = 4, 2048
N_CORES = 8
HG_HEADS = 6            # heads per core (3 pairs)
HG_DIM = HG_HEADS * HEAD_DIM   # 384
QKV_W = 3 * HG_DIM      # 1152 qkv columns per core
N_PAIRS = 3
ST = S // 128           # 16 seq tiles of 128
NG = S // 512           # 4 seq groups of 512

# last run's BassKernelResults (test.py reads this for HW timing / traces)
LAST_RESULTS = None
_PROGRAM = None


def _build_program():
    """Build (once) the SPMD Bass program run identically on all 8 cores."""
    import concourse.bacc as bacc
    import concourse.tile as tile
    from concourse import mybir

    F32R = mybir.dt.float32r
    F32 = mybir.dt.float32
    BF16 = mybir.dt.bfloat16
    AF = mybir.ActivationFunctionType

    nc = bacc.Bacc(None, target_bir_lowering=False)
    xt_d = nc.declare_dram_parameter("xt", [N_EMBD, S], BF16, isOutput=False)
    # host-packed weight blocks, contiguous in consumption order:
    # [q0 | k0 | v(all pairs) | q1 | k1 | q2 | k2], each block k-major
    wblk_d = nc.declare_dram_parameter("w_blk", [128, 6 * QKV_W], BF16, isOutput=False)
    bqk_d = nc.declare_dram_parameter("b_qk", [768], F32, isOutput=False)
    ones_d = nc.declare_dram_parameter("ones", [1, 128], F32R, isOutput=False)
    wproj_d = nc.declare_dram_parameter("w_proj", [HG_DIM, N_EMBD], BF16, isOutput=False)
    y_d = nc.declare_dram_parameter("y", [S, N_EMBD], F32, isOutput=True)

    with tile.TileContext(nc) as tc:
        from contextlib import ExitStack

        with ExitStack() as outer:
            consts = outer.enter_context(tc.tile_pool(name="consts", bufs=1))
            ones_row = consts.tile([1, 128], F32R)
            nc.sync.dma_start(out=ones_row[:], in_=ones_d[:])
            bias_qk = consts.tile([128, 6], F32)      # col m: b_qk[128m:128m+128]
            nc.sync.dma_start(
                out=bias_qk[:], in_=bqk_d[0:768].rearrange("(m p) -> p m", p=128)
            )

            # ---- persistent activations/weights in SBUF ----
            big = outer.enter_context(tc.tile_pool(name="big", bufs=1))
            xT = big.tile([128, 6 * S], BF16)      # [emb-part, k-tile*2048+seq]
            qkT = big.tile([128, 6 * S], BF16)     # m=0..2 qT pairs, m=3..5 kT pairs
            # per k-tile: 6 heads x (64 v-cols + a ones col for the softmax
            # denominator) -> P@V and row-sums come from one M=65 matmul
            v_all = big.tile([128, ST * 390], BF16)  # [seq, t*390 + 65h + d]
            nc.gpsimd.memset(v_all[:], 1.0)
            attnT = big.tile([128, N_PAIRS * S], BF16)  # [pair d, pair*2048+seq]
            w_all = big.tile([128, 6 * QKV_W], BF16)
            w_proj = big.tile([128, N_PAIRS * N_EMBD], BF16)

            # preload the exp table set while DMAs stream in
            dummy = consts.tile([1, 128], F32)
            nc.scalar.activation(dummy[:], ones_row[:].bitcast(F32), AF.Exp,
                                 bias=0.0, scale=0.125)
            # seeded garbage tile for PE warm-keeper matmuls
            garbage = consts.tile([128, 512], BF16)
            nc.sync.dma_start(out=garbage[0:1, 0:128],
                              in_=ones_d[:].bitcast(BF16)[:, 0:128])

            # input DMAs, in consumption order: pair-0 q/k weights, the g0
            # columns of xT (unblocks the first qk block), v weights, the
            # rest of xT, remaining q/k weights.  All DRAM reads contiguous:
            # x pre-transposed and weights pre-packed by the host.
            xT_v = xT[:].rearrange("p (k s) -> p k s", k=6)
            w_view = w_all[:].rearrange("p (k c) -> p k c", k=6)

            def dma_w(blk_off, width, c0):
                nc.sync.dma_start(
                    out=w_view[:, :, c0:c0 + width],
                    in_=wblk_d[:, blk_off:blk_off + 6 * width].rearrange(
                        "p (k c) -> p k c", k=6),
                )

            def dma_x(gq):
                for k in range(6):
                    nc.sync.dma_start(
                        out=xT_v[:, k, gq * 512:(gq + 1) * 512],
                        in_=xt_d[k * 128:(k + 1) * 128, gq * 512:(gq + 1) * 512],
                    )

            dma_w(0, 128, 0)        # q pair 0
            dma_w(768, 128, 384)    # k pair 0
            dma_x(0)
            dma_w(1536, 384, 768)   # v columns, all pairs
            dma_x(1)
            dma_x(2)
            dma_x(3)
            dma_w(3840, 128, 128)   # q pair 1
            dma_w(4608, 128, 512)   # k pair 1
            dma_w(5376, 128, 256)   # q pair 2
            dma_w(6144, 128, 640)   # k pair 2
            for p in range(N_PAIRS):
                nc.sync.dma_start(
                    out=w_proj[:, p * N_EMBD:(p + 1) * N_EMBD],
                    in_=wproj_d[p * 128:(p + 1) * 128, :],
                )

            # ---- filler blocks: qkv projections + output projection ----
            fill = outer.enter_context(
                tc.tile_pool(name="fill", bufs=1, space="PSUM"))
            ys_pool = outer.enter_context(tc.tile_pool(name="ys", bufs=2))
            v_v = v_all[:].rearrange("p (t h d) -> p t h d", t=ST, h=6)

            def qk_block(m, g):
                # qT (m=pair) / kT (m=3+pair) for one 512-col seq group
                ps = fill.tile([128, 512], F32, tag="fill")
                for k in range(6):
                    nc.tensor.matmul(
                        ps[:],
                        w_all[:, k * QKV_W + m * 128:k * QKV_W + (m + 1) * 128],
                        xT_v[:, k, g * 512:(g + 1) * 512],
                        start=(k == 0), stop=(k == 5),
                    )
                nc.vector.tensor_scalar_add(
                    qkT[:, m * S + g * 512:m * S + (g + 1) * 512],
                    ps[:], bias_qk[:, m:m + 1],
                )

            def v_block(pr, t, npr=1):
                # v (npr head pairs from pr) for one 128-row seq tile
                ps = fill.tile([128, 512], F32, tag="fill")
                vc = 768 + pr * 128
                w = npr * 128
                for k in range(6):
                    nc.tensor.matmul(
                        ps[:, 0:w],
                        xT_v[:, k, t * 128:(t + 1) * 128],
                        w_all[:, k * QKV_W + vc:k * QKV_W + vc + w],
                        start=(k == 0), stop=(k == 5),
                    )
                nc.vector.tensor_copy(
                    v_v[:, t, 2 * pr:2 * pr + 2 * npr, 0:64],
                    ps[:, 0:w].rearrange("p (h d) -> p h d", h=2 * npr),
                )

            def proj_block(t, half, pool=None, on_act=False):
                # y[:, 384*half : 384*(half+1)] for one 128-row seq tile
                ps = (pool or fill).tile([128, 384], F32, tag="fill")
                h0 = 384 * half
                for p in range(N_PAIRS):
                    nc.tensor.matmul(
                        ps[:],
                        attnT[:, p * S + t * 128:p * S + (t + 1) * 128],
                        w_proj[:, p * N_EMBD + h0:p * N_EMBD + h0 + 384],
                        start=(p == 0), stop=(p == N_PAIRS - 1),
                    )
                ys = ys_pool.tile([128, 384], F32)
                if on_act:   # tail: ACT is idle, DVE is the serializer
                    nc.scalar.copy(ys[:], ps[:])
                else:
                    nc.vector.tensor_copy(ys[:], ps[:])
                nc.sync.dma_start(
                    out=y_d[t * 128:(t + 1) * 128, h0:h0 + 384], in_=ys[:])

            # filler emission schedule: blocks spread across the j-loops of
            # each (pair, g) attention group, ordered so every block lands
            # before its consumer group starts.
            spread = {
                (0, 0): [(v_block, 0, 0, 2), (v_block, 0, 1, 2),
                         (v_block, 0, 2, 2), (v_block, 0, 3, 2),
                         (qk_block, 0, 1), (qk_block, 3, 1)],
                (0, 1): [(v_block, 0, 4, 2), (v_block, 0, 5, 2),
                         (v_block, 0, 6, 2), (v_block, 0, 7, 2),
                         (qk_block, 0, 2), (qk_block, 3, 2)],
                (0, 2): [(v_block, 0, 8, 2), (v_block, 0, 9, 2),
                         (v_block, 0, 10, 2), (v_block, 0, 11, 2),
                         (qk_block, 0, 3), (qk_block, 3, 3),
                         (qk_block, 1, 0), (qk_block, 4, 0)],
                (0, 3): [(v_block, 0, 12, 2), (v_block, 0, 13, 2),
                         (v_block, 0, 14, 2), (v_block, 0, 15, 2),
                         (qk_block, 1, 1), (qk_block, 4, 1)],
                (1, 0): [(qk_block, 1, 2), (qk_block, 4, 2),
                         (v_block, 2, 0), (v_block, 2, 1)],
                (1, 1): [(qk_block, 1, 3), (qk_block, 4, 3),
                         (v_block, 2, 2), (v_block, 2, 3),
                         (v_block, 2, 4), (v_block, 2, 5),
                         (qk_block, 2, 0), (qk_block, 5, 0)],
                (1, 2): [(v_block, 2, 6), (v_block, 2, 7),
                         (v_block, 2, 8), (v_block, 2, 9),
                         (qk_block, 2, 1), (qk_block, 5, 1)],
                (1, 3): [(v_block, 2, 10), (v_block, 2, 11),
                         (v_block, 2, 12), (v_block, 2, 13),
                         (v_block, 2, 14), (v_block, 2, 15),
                         (qk_block, 2, 2), (qk_block, 5, 2)],
                (2, 0): [(qk_block, 2, 3), (qk_block, 5, 3)],
                (2, 1): [(proj_block, t, h) for t in range(4) for h in (0, 1)],
                (2, 2): [(proj_block, t, h) for t in range(4, 8) for h in (0, 1)],
                (2, 3): [(proj_block, t, h) for t in range(8, 12) for h in (0, 1)],
            }

            # ---- head: first pair's g=0 q/k (v blocks are (0,0) fillers).
            # Dummy matmuls on the early-seeded garbage tile warm the PE HAM
            # clock gate (1.2 -> 2.4 GHz) during the input-DMA wait.
            warmh = fill.tile([128, 512], F32, tag="fill")
            for i in range(8):
                nc.tensor.matmul(warmh[:], garbage[:, 0:128], garbage[:],
                                 start=True, stop=True)
            qk_block(0, 0)
            qk_block(3, 0)

            # ---- attention: ACT-bound j-loops with PE filler interleave ----
            with tc.tile_pool(name="stps", bufs=2, space="PSUM") as stps, \
                 tc.tile_pool(name="avps", bufs=3, space="PSUM") as avps, \
                 tc.tile_pool(name="ptp", bufs=5) as ptp, \
                 tc.tile_pool(name="rcp", bufs=2) as rcp, \
                 tc.tile_pool(name="bcp", bufs=2) as bcp, \
                 tc.tile_pool(name="avcp", bufs=2) as avcp, \
                 tc.tile_pool(name="shtmp", bufs=2) as shtmp:
                for pair in range(N_PAIRS):
                    q0 = pair * S          # qT pair tile offset in qkT
                    k0 = (3 + pair) * S    # kT pair tile offset
                    for g in range(NG):
                        av0 = avps.tile([65, 512], F32, tag="av")
                        av1 = avps.tile([65, 512], F32, tag="av")
                        avs = (av0, av1)
                        njt = 4 * g + 4
                        fills = list(spread[(pair, g)])
                        nfill = len(fills)
                        avq = []  # software-pipeline AV two j behind
                        for j in range(njt):
                            diag_r = j - 4 * g   # >=0 on diagonal tiles
                            c0 = 128 * diag_r if diag_r >= 0 else 0
                            st = stps.tile([128, 1024], F32, tag="st")
                            pt = ptp.tile([128, 1024], BF16, tag="pt")
                            # row-packed scores: both heads concurrently
                            nc.tensor.matmul(
                                st[:, c0:512],
                                qkT[0:64, k0 + j * 128:k0 + (j + 1) * 128],
                                qkT[0:64, q0 + g * 512 + c0:q0 + (g + 1) * 512],
                                start=True, stop=True, tile_position=(0, 0),
                            )
                            nc.tensor.matmul(
                                st[:, 512 + c0:1024],
                                qkT[64:128, k0 + j * 128:k0 + (j + 1) * 128],
                                qkT[64:128, q0 + g * 512 + c0:q0 + (g + 1) * 512],
                                start=True, stop=True, tile_position=(64, 0),
                            )
                            # exp(S/8) over both heads' valid columns
                            nc.scalar.activation(
                                pt[:, c0:1024], st[:, c0:1024], AF.Exp,
                                bias=0.0, scale=0.125,
                            )
                            if diag_r >= 0:
                                # zero the strictly-lower (k>q) triangle of
                                # both heads in one 3D-batched op
                                p3 = pt[:].rearrange("p (h s) -> p h s", h=2)
                                nc.gpsimd.affine_select(
                                    out=p3[:, :, c0:c0 + 128],
                                    in_=p3[:, :, c0:c0 + 128],
                                    compare_op=mybir.AluOpType.is_ge,
                                    fill=0.0, base=0,
                                    pattern=[[0, 2], [1, 128]],
                                    channel_multiplier=-1,
                                )
                            avq.append((j, c0, pt))
                            if len(avq) > 2:
                                _emit_av(nc, avs, v_all, pair, avq.pop(0), njt)
                            # PE filler between j iterations (evenly, but
                            # none on the last iteration so the group
                            # boundary's exp -> scores chain stays clean)
                            jlast = max(1, njt - 1)
                            left = max(0, jlast - 1 - j)
                            while fills and len(fills) > (nfill * left) // jlast:
                                blk = fills.pop(0)
                                blk[0](*blk[1:])
                        for prev in avq:
                            _emit_av(nc, avs, v_all, pair, prev, njt)

                        # ---- normalization tail ----
                        cols = slice(pair * S + g * 512, pair * S + (g + 1) * 512)
                        for h in range(2):
                            # den row 64 -> partition 0 (plain copies handle
                            # the shift; reciprocal_approx_fast does NOT work
                            # on base-partition-64 APs)
                            rc = rcp.tile([1, 512], F32, tag="rc")
                            nc.vector.tensor_copy(rc[:], avs[h][64:65, :])
                            nc.vector.reciprocal_approx_fast(rc[:], rc[:])
                            bc = bcp.tile([64, 512], F32, tag="bc")
                            nc.gpsimd.partition_broadcast(bc[:], rc[:])
                            if h == 0:
                                nc.vector.tensor_mul(
                                    attnT[0:64, cols], avs[h][0:64, :], bc[:])
                            else:
                                # DVE lanes are partition-locked: odd head's
                                # rows 64-127 go via an SBUF bounce + DMA
                                tmp = shtmp.tile([64, 512], BF16, tag="sh")
                                nc.vector.tensor_mul(
                                    tmp[:], avs[h][0:64, :], bc[:])
                                nc.sync.dma_start(out=attnT[64:128, cols],
                                                  in_=tmp[:])
                        if pair == 2 and g == 3:
                            # bridge the PE idle of the final normalization
                            # chain so the tail projection runs at 2.4 GHz
                            wps = fill.tile([128, 512], F32, tag="fill")
                            for i in range(12):
                                nc.tensor.matmul(
                                    wps[:, 0:384], garbage[0:64, 0:128],
                                    tmp[:, 0:384], start=True, stop=True)

            # ---- remaining output projection (PSUM free: deep-buffer it).
            # Dummy matmuls bridge the PE idle window during the last
            # normalization so the HAM clock stays at 2.4 GHz for the tail.
            with tc.tile_pool(name="tailp", bufs=3, space="PSUM") as tailp:
                warm = tailp.tile([128, 384], F32, tag="warm")
                for i in range(14):
                    nc.tensor.matmul(warm[:], garbage[:, 0:128],
                                     garbage[:, 0:384], start=True, stop=True)
                for t in range(12, ST):
                    proj_block(t, 0, pool=tailp, on_act=True)
                    proj_block(t, 1, pool=tailp, on_act=True)

    nc.compile()
    return nc


def _emit_av(nc, avs, v_all, pair, prev, njt):
    # [attn-out^T ; denominators] accumulated over k-tiles; ones columns in
    # v_all put the denominators in output row 64.
    j, c0, pt = prev
    for h in range(2):
        hl = 2 * pair + h
        nc.tensor.matmul(
            avs[h][0:65, c0:512],
            v_all[:, j * 390 + hl * 65:j * 390 + hl * 65 + 65],
            pt[:, h * 512 + c0:(h + 1) * 512],
            start=(j == 0), stop=(j == njt - 1),
        )


def _numpy_fallback(x, mask, W_attn, b_attn, W_proj, b_proj):
    qkv = x @ W_attn + b_attn
    q, k, v = np.split(qkv, 3, axis=-1)

    def heads(t):
        return t.reshape(B, S, N_HEAD, HEAD_DIM).transpose(0, 2, 1, 3)

    q, k, v = heads(q), heads(k), heads(v)
    attn = np.einsum("bhqd,bhkd->bhqk", q, k) / np.sqrt(np.float32(HEAD_DIM))
    attn = attn + mask * (-1e9)
    attn = attn - attn.max(axis=-1, keepdims=True)
    attn = np.exp(attn)
    attn = attn / attn.sum(axis=-1, keepdims=True)
    out = np.einsum("bhqk,bhkd->bhqd", attn, v)
    out = out.transpose(0, 2, 1, 3).reshape(B, S, N_EMBD)
    return (out @ W_proj + b_proj).astype(np.float32)


def kernel(x, mask, W_attn, b_attn, W_proj, b_proj):
    global LAST_RESULTS, _PROGRAM
    x = np.asarray(x, dtype=np.float32)
    mask = np.asarray(mask, dtype=np.float32)
    W_attn = np.asarray(W_attn, dtype=np.float32)
    b_attn = np.asarray(b_attn, dtype=np.float32)
    W_proj = np.asarray(W_proj, dtype=np.float32)
    b_proj = np.asarray(b_proj, dtype=np.float32)

    # the kernel exploits causal structure; verify the mask actually is causal
    causal = 1.0 - np.tril(np.ones((S, S), dtype=np.float32))
    if mask.shape != (1, 1, S, S) or not np.array_equal(mask[0, 0], causal):
        return _numpy_fallback(x, mask, W_attn, b_attn, W_proj, b_proj)

    from concourse.bass_utils import run_bass_kernel_spmd

    if _PROGRAM is None:
        _PROGRAM = _build_program()

    in_maps = make_in_maps(x, W_attn, b_attn, W_proj)

    trace = bool(int(os.environ.get("ATTN_KERNEL_TRACE", "0")))
    res = run_bass_kernel_spmd(_PROGRAM, in_maps, list(range(N_CORES)), trace=trace)
    LAST_RESULTS = res

    y = np.zeros((B, S, N_EMBD), dtype=np.float32)
    for c in range(N_CORES):
        y[c // 2] += res.results[c]["y"]
    # softmax rows sum to 1: v-bias contributes b_v @ W_proj, a constant
    y += b_attn[1536:] @ W_proj + b_proj
    return y


def _pack_w_blocks(Wq, Wk, Wv):
    """[q0 | k0 | v(all) | q1 | k1 | q2 | k2], each block k-major [128, 6*w]."""
    def blk(W, m0, m1):
        return np.concatenate(
            [W[k * 128:(k + 1) * 128, m0 * 128:m1 * 128] for k in range(6)],
            axis=1)
    return np.ascontiguousarray(np.concatenate(
        [blk(Wq, 0, 1), blk(Wk, 0, 1), blk(Wv, 0, 3), blk(Wq, 1, 2),
         blk(Wk, 1, 2), blk(Wq, 2, 3), blk(Wk, 2, 3)], axis=1))


def make_in_maps(x, W_attn, b_attn, W_proj):
    bf16 = ml_dtypes.bfloat16
    in_maps = []
    for c in range(N_CORES):
        b, hg = divmod(c, 2)
        o = HG_DIM * hg
        in_maps.append({
            "xt": np.ascontiguousarray(x[b].astype(bf16).T),
            "w_blk": _pack_w_blocks(
                W_attn[:, o:o + HG_DIM].astype(bf16),
                W_attn[:, 768 + o:768 + o + HG_DIM].astype(bf16),
                W_attn[:, 1536 + o:1536 + o + HG_DIM].astype(bf16)),
            "b_qk": np.ascontiguousarray(np.concatenate(
                [b_attn[o:o + HG_DIM], b_attn[768 + o:768 + o + HG_DIM]])),
            "w_proj": np.ascontiguousarray(W_proj[o:o + HG_DIM, :].astype(bf16)),
            "ones": np.ones((1, 128), dtype=np.float32),
        })
    return in_maps


# revision 47
# speedup vs baseline: 1.1947x; 1.1947x over previous
"""Trainium2 Bass kernel for a 12-head causal attention block (GPT-2 style).

Problem: x:[4,2048,768] -> qkv = x@W_attn+b_attn, causal softmax attention
(12 heads, d=64), out @ W_proj + b_proj.

Sharding over 8 NeuronCores: core c handles batch b=c//2 (data parallel) and
head-group hg=c%2 (6 heads, tensor parallel on the qkv columns / proj rows).
Each core returns a partial projection output; the host sums the two
head-group partials per batch and adds the output bias (b_proj plus the
b_v@W_proj term: softmax rows sum to 1, so the v-bias contributes a constant
vector to the attention output and is folded host-side).

Per-core dataflow (inputs bf16; matmul accumulation fp32):
  - xT [emb, seq] comes straight from DRAM via DMA-transpose (bf16).
  - qkT = W-tiles.T @ xT -> qT,kT per head-pair [128,2048] (even head rows
    0-63, odd head rows 64-127); v in natural [seq, d] layout interleaved
    with ones columns (ones give the softmax denominators for free in the
    P@V matmul's 65th output row).
  - scores S^T[k,q] per 128k x 512q block: the two heads of a pair run
    ROW-PACKED (tile_position (0,0)/(64,0)) and execute concurrently in the
    PE array; one ACT exp call covers both heads.  Upper-triangle blocks are
    skipped; diagonal-crossing triangles zeroed post-exp with one 3D-batched
    gpsimd affine_select.  No max-subtraction needed (|scores/8| small).
  - P@V accumulates [attn-out^T ; den] in PSUM over k-tiles (M=65).
  - normalization: DVE reciprocal_approx_fast on the den row (~18 bits,
    5x faster than exact), gpsimd partition_broadcast to 64 rows, DVE
    multiply -> attnT (bf16).  Odd head bounced to rows 64-127 via DMA.
  - proj: y[128q,768] accumulated over the 3 head-pair k-tiles in 384-col
    chunks.
  - PE saturation: the attention j-loop is ACT(exp)-bound, so the qkv
    projection matmuls for LATER pairs and the output projection are emitted
    as filler blocks interleaved between j-iterations, keeping the PE busy
    (and HAM-warm) throughout instead of running phases serially.
"""

import os
import ml_dtypes
import numpy as np

N_HEAD = 12
N_EMBD = 768
HEAD_DIM = 64
B, S = 4, 2048
N_CORES = 8
HG_HEADS = 6            # heads per core (3 pairs)
HG_DIM = HG_HEADS * HEAD_DIM   # 384
QKV_W = 3 * HG_DIM      # 1152 qkv columns per core
N_PAIRS = 3
ST = S // 128           # 16 seq tiles of 128
NG = S // 512           # 4 seq groups of 512

# last run's BassKernelResults (test.py reads this for HW timing / traces)
LAST_RESULTS = None
_PROGRAM = None


def _build_program():
    """Build (once) the SPMD Bass program run identically on all 8 cores."""
    import concourse.bacc as bacc
    import concourse.tile as tile
    from concourse import mybir

    F32R = mybir.dt.float32r
    F32 = mybir.dt.float32
    BF16 = mybir.dt.bfloat16
    AF = mybir.ActivationFunctionType

    nc = bacc.Bacc(None, target_bir_lowering=False)
    xt_d = nc.declare_dram_parameter("xt", [N_EMBD, S], BF16, isOutput=False)
    # host-packed weight blocks, contiguous in consumption order:
    # [q0 | k0 | v(all pairs) | q1 | k1 | q2 | k2], each block k-major
    wblk_d = nc.declare_dram_parameter("w_blk", [128, 6 * QKV_W], BF16, isOutput=False)
    bqk_d = nc.declare_dram_parameter("b_qk", [768], F32, isOutput=False)
    ones_d = nc.declare_dram_parameter("ones", [1, 128], F32R, isOutput=False)
    wproj_d = nc.declare_dram_parameter("w_proj", [HG_DIM, N_EMBD], BF16, isOutput=False)
    y_d = nc.declare_dram_parameter("y", [S, N_EMBD], F32, isOutput=True)

    with tile.TileContext(nc) as tc:
        from contextlib import ExitStack

        with ExitStack() as outer:
            consts = outer.enter_context(tc.tile_pool(name="consts", bufs=1))
            ones_row = consts.tile([1, 128], F32R)
            nc.sync.dma_start(out=ones_row[:], in_=ones_d[:])
            bias_qk = consts.tile([128, 6], F32)      # col m: b_qk[128m:128m+128]
            nc.sync.dma_start(
                out=bias_qk[:], in_=bqk_d[0:768].rearrange("(m p) -> p m", p=128)
            )

            # ---- persistent activations/weights in SBUF ----
            big = outer.enter_context(tc.tile_pool(name="big", bufs=1))
            xT = big.tile([128, 6 * S], BF16)      # [emb-part, k-tile*2048+seq]
            qkT = big.tile([128, 6 * S], BF16)     # m=0..2 qT pairs, m=3..5 kT pairs
            # per k-tile: 6 heads x (64 v-cols + a ones col for the softmax
            # denominator) -> P@V and row-sums come from one M=65 matmul
            v_all = big.tile([128, ST * 390], BF16)  # [seq, t*390 + 65h + d]
            nc.gpsimd.memset(v_all[:], 1.0)
            attnT = big.tile([128, N_PAIRS * S], BF16)  # [pair d, pair*2048+seq]
            w_all = big.tile([128, 6 * QKV_W], BF16)
            w_proj = big.tile([128, N_PAIRS * N_EMBD], BF16)

            # preload the exp table set while DMAs stream in
            dummy = consts.tile([1, 128], F32)
            nc.scalar.activation(dummy[:], ones_row[:].bitcast(F32), AF.Exp,
                                 bias=0.0, scale=0.125)
            # seeded garbage tile for PE warm-keeper matmuls
            garbage = consts.tile([128, 512], BF16)
            nc.sync.dma_start(out=garbage[0:1, 0:128],
                              in_=ones_d[:].bitcast(BF16)[:, 0:128])

            # input DMAs, in consumption order: pair-0 q/k weights, the g0
            # columns of xT (unblocks the first qk block), v weights, the
            # rest of xT, remaining q/k weights.  All DRAM reads contiguous:
            # x pre-transposed and weights pre-packed by the host.
            xT_v = xT[:].rearrange("p (k s) -> p k s", k=6)
            w_view = w_all[:].rearrange("p (k c) -> p k c", k=6)

            def dma_w(blk_off, width, c0):
                nc.sync.dma_start(
                    out=w_view[:, :, c0:c0 + width],
                    in_=wblk_d[:, blk_off:blk_off + 6 * width].rearrange(
                        "p (k c) -> p k c", k=6),
                )

            def dma_x(gq):
                for k in range(6):
                    nc.sync.dma_start(
                        out=xT_v[:, k, gq * 512:(gq + 1) * 512],
                        in_=xt_d[k * 128:(k + 1) * 128, gq * 512:(gq + 1) * 512],
                    )

            dma_w(0, 128, 0)        # q pair 0
            dma_w(768, 128, 384)    # k pair 0
            dma_x(0)
            dma_w(1536, 384, 768)   # v columns, all pairs
            dma_x(1)
            dma_x(2)
            dma_x(3)
            dma_w(3840, 128, 128)   # q pair 1
            dma_w(4608, 128, 512)   # k pair 1
            dma_w(5376, 128, 256)   # q pair 2
            dma_w(6144, 128, 640)   # k pair 2
            for p in range(N_PAIRS):
                nc.sync.dma_start(
                    out=w_proj[:, p * N_EMBD:(p + 1) * N_EMBD],
                    in_=wproj_d[p * 128:(p + 1) * 128, :],
                )

            # ---- filler blocks: qkv projections + output projection ----
            fill = outer.enter_context(
                tc.tile_pool(name="fill", bufs=1, space="PSUM"))
            ys_pool = outer.enter_context(tc.tile_pool(name="ys", bufs=2))
            v_v = v_all[:].rearrange("p (t h d) -> p t h d", t=ST, h=6)

            def qk_block(m, g):
                # qT (m=pair) / kT (m=3+pair) for one 512-col seq group
                ps = fill.tile([128, 512], F32, tag="fill")
                for k in range(6):
                    nc.tensor.matmul(
                        ps[:],
                        w_all[:, k * QKV_W + m * 128:k * QKV_W + (m + 1) * 128],
                        xT_v[:, k, g * 512:(g + 1) * 512],
                        start=(k == 0), stop=(k == 5),
                    )
                nc.vector.tensor_scalar_add(
                    qkT[:, m * S + g * 512:m * S + (g + 1) * 512],
                    ps[:], bias_qk[:, m:m + 1],
                )

            def v_block(pr, t, npr=1):
                # v (npr head pairs from pr) for one 128-row seq tile
                ps = fill.tile([128, 512], F32, tag="fill")
                vc = 768 + pr * 128
                w = npr * 128
                for k in range(6):
                    nc.tensor.matmul(
                        ps[:, 0:w],
                        xT_v[:, k, t * 128:(t + 1) * 128],
                        w_all[:, k * QKV_W + vc:k * QKV_W + vc + w],
                        start=(k == 0), stop=(k == 5),
                    )
                nc.vector.tensor_copy(
                    v_v[:, t, 2 * pr:2 * pr + 2 * npr, 0:64],
                    ps[:, 0:w].rearrange("p (h d) -> p h d", h=2 * npr),
                )

            def proj_block(t, half, pool=None, on_act=False):
                # y[:, 384*half : 384*(half+1)] for one 128-row seq tile
                ps = (pool or fill).tile([128, 384], F32, tag="fill")
                h0 = 384 * half
                for p in range(N_PAIRS):
                    nc.tensor.matmul(
                        ps[:],
                        attnT[:, p * S + t * 128:p * S + (t + 1) * 128],
                        w_proj[:, p * N_EMBD + h0:p * N_EMBD + h0 + 384],
                        start=(p == 0), stop=(p == N_PAIRS - 1),
                    )
                ys = ys_pool.tile([128, 384], F32)
                if on_act:   # tail: ACT is idle, DVE is the serializer
                    nc.scalar.copy(ys[:], ps[:])
                else:
                    nc.vector.tensor_copy(ys[:], ps[:])
                nc.sync.dma_start(
                    out=y_d[t * 128:(t + 1) * 128, h0:h0 + 384], in_=ys[:])

            # filler emission schedule: blocks spread across the j-loops of
            # each (pair, g) attention group, ordered so every block lands
            # before its consumer group starts.
            spread = {
                (0, 0): [(v_block, 0, 0, 2), (v_block, 0, 1, 2),
                         (v_block, 0, 2, 2), (v_block, 0, 3, 2),
                         (qk_block, 0, 1), (qk_block, 3, 1)],
                (0, 1): [(v_block, 0, 4, 2), (v_block, 0, 5, 2),
                         (v_block, 0, 6, 2), (v_block, 0, 7, 2),
                         (qk_block, 0, 2), (qk_block, 3, 2)],
                (0, 2): [(v_block, 0, 8, 2), (v_block, 0, 9, 2),
                         (v_block, 0, 10, 2), (v_block, 0, 11, 2),
                         (qk_block, 0, 3), (qk_block, 3, 3),
                         (qk_block, 1, 0), (qk_block, 4, 0)],
                (0, 3): [(v_block, 0, 12, 2), (v_block, 0, 13, 2),
                         (v_block, 0, 14, 2), (v_block, 0, 15, 2),
                         (qk_block, 1, 1), (qk_block, 4, 1)],
                (1, 0): [(qk_block, 1, 2), (qk_block, 4, 2),
                         (v_block, 2, 0), (v_block, 2, 1)],
                (1, 1): [(qk_block, 1, 3), (qk_block, 4, 3),
                         (v_block, 2, 2), (v_block, 2, 3),
                         (v_block, 2, 4), (v_block, 2, 5),
                         (qk_block, 2, 0), (qk_block, 5, 0)],
                (1, 2): [(v_block, 2, 6), (v_block, 2, 7),
                         (v_block, 2, 8), (v_block, 2, 9),
                         (qk_block, 2, 1), (qk_block, 5, 1)],
                (1, 3): [(v_block, 2, 10), (v_block, 2, 11),
                         (v_block, 2, 12), (v_block, 2, 13),
                         (v_block, 2, 14), (v_block, 2, 15),
                         (qk_block, 2, 2), (qk_block, 5, 2)],
                (2, 0): [(qk_block, 2, 3), (qk_block, 5, 3)],
                (2, 1): [(proj_block, t, h) for t in range(4) for h in (0, 1)],
                (2, 2): [(proj_block, t, h) for t in range(4, 8) for h in (0, 1)],
                (2, 3): [(proj_block, t, h) for t in range(8, 12) for h in (0, 1)],
            }

            # ---- head: first pair's g=0 q/k (v blocks are (0,0) fillers).
            # Dummy matmuls on the early-seeded garbage tile warm the PE HAM
            # clock gate (1.2 -> 2.4 GHz) during the input-DMA wait.
            warmh = fill.tile([128, 512], F32, tag="fill")
            for i in range(8):
                nc.tensor.matmul(warmh[:], garbage[:, 0:128], garbage[:],
                                 start=True, stop=True)
            qk_block(0, 0)
            qk_block(3, 0)

            # ---- attention: ACT-bound j-loops with PE filler interleave ----
            with tc.tile_pool(name="stps", bufs=2, space="PSUM") as stps, \
                 tc.tile_pool(name="avps", bufs=3, space="PSUM") as avps, \
                 tc.tile_pool(name="ptp", bufs=5) as ptp, \
                 tc.tile_pool(name="rcp", bufs=2) as rcp, \
                 tc.tile_pool(name="bcp", bufs=2) as bcp, \
                 tc.tile_pool(name="avcp", bufs=2) as avcp, \
                 tc.tile_pool(name="shtmp", bufs=2) as shtmp:
                for pair in range(N_PAIRS):
                    q0 = pair * S          # qT pair tile offset in qkT
                    k0 = (3 + pair) * S    # kT pair tile offset
                    for g in range(NG):
                        av0 = avps.tile([65, 512], F32, tag="av")
                        av1 = avps.tile([65, 512], F32, tag="av")
                        avs = (av0, av1)
                        njt = 4 * g + 4
                        fills = list(spread[(pair, g)])
                        nfill = len(fills)
                        avq = []  # software-pipeline AV two j behind
                        for j in range(njt):
                            diag_r = j - 4 * g   # >=0 on diagonal tiles
                            c0 = 128 * diag_r if diag_r >= 0 else 0
                            st = stps.tile([128, 1024], F32, tag="st")
                            pt = ptp.tile([128, 1024], BF16, tag="pt")
                            # row-packed scores: both heads concurrently
                            nc.tensor.matmul(
                                st[:, c0:512],
                                qkT[0:64, k0 + j * 128:k0 + (j + 1) * 128],
                                qkT[0:64, q0 + g * 512 + c0:q0 + (g + 1) * 512],
                                start=True, stop=True, tile_position=(0, 0),
                            )
                            nc.tensor.matmul(
                                st[:, 512 + c0:1024],
                                qkT[64:128, k0 + j * 128:k0 + (j + 1) * 128],
                                qkT[64:128, q0 + g * 512 + c0:q0 + (g + 1) * 512],
                                start=True, stop=True, tile_position=(64, 0),
                            )
                            # exp(S/8) over both heads' valid columns
                            nc.scalar.activation(
                                pt[:, c0:1024], st[:, c0:1024], AF.Exp,
                                bias=0.0, scale=0.125,
                            )
                            if diag_r >= 0:
                                # zero the strictly-lower (k>q) triangle of
                                # both heads in one 3D-batched op
                                p3 = pt[:].rearrange("p (h s) -> p h s", h=2)
                                nc.gpsimd.affine_select(
                                    out=p3[:, :, c0:c0 + 128],
                                    in_=p3[:, :, c0:c0 + 128],
                                    compare_op=mybir.AluOpType.is_ge,
                                    fill=0.0, base=0,
                                    pattern=[[0, 2], [1, 128]],
                                    channel_multiplier=-1,
                                )
                            avq.append((j, c0, pt))
                            if len(avq) > 3:
                                _emit_av(nc, avs, v_all, pair, avq.pop(0), njt)
                            # PE filler between j iterations
                            while fills and len(fills) > (nfill * (njt - 1 - j)) // njt:
                                blk = fills.pop(0)
                                blk[0](*blk[1:])
                        for prev in avq:
                            _emit_av(nc, avs, v_all, pair, prev, njt)

                        # ---- normalization tail ----
                        cols = slice(pair * S + g * 512, pair * S + (g + 1) * 512)
                        for h in range(2):
                            # den row 64 -> partition 0 (plain copies handle
                            # the shift; reciprocal_approx_fast does NOT work
                            # on base-partition-64 APs)
                            rc = rcp.tile([1, 512], F32, tag="rc")
                            nc.vector.tensor_copy(rc[:], avs[h][64:65, :])
                            nc.vector.reciprocal_approx_fast(rc[:], rc[:])
                            bc = bcp.tile([64, 512], F32, tag="bc")
                            nc.gpsimd.partition_broadcast(bc[:], rc[:])
                            if h == 0:
                                nc.vector.tensor_mul(
                                    attnT[0:64, cols], avs[h][0:64, :], bc[:])
                            else:
                                # DVE lanes are partition-locked: odd head's
                                # rows 64-127 go via an SBUF bounce + DMA
                                tmp = shtmp.tile([64, 512], BF16, tag="sh")
                                nc.vector.tensor_mul(
                                    tmp[:], avs[h][0:64, :], bc[:])
                                nc.sync.dma_start(out=attnT[64:128, cols],
                                                  in_=tmp[:])
                        if pair == 2 and g == 3:
                            # bridge the PE idle of the final normalization
                            # chain so the tail projection runs at 2.4 GHz
                            wps = fill.tile([128, 512], F32, tag="fill")
                            for i in range(12):
                                nc.tensor.matmul(
                                    wps[:, 0:384], garbage[0:64, 0:128],
                                    tmp[:, 0:384], start=True, stop=True)

            # ---- remaining output projection (PSUM free: deep-buffer it).
            # Dummy matmuls bridge the PE idle window during the last
            # normalization so the HAM clock stays at 2.4 GHz for the tail.
            with tc.tile_pool(name="tailp", bufs=3, space="PSUM") as tailp:
                warm = tailp.tile([128, 384], F32, tag="warm")
                for i in range(14):
                    nc.tensor.matmul(warm[:], garbage[:, 0:128],
                                     garbage[:, 0:384], start=True, stop=True)
                for t in range(12, ST):
                    proj_block(t, 0, pool=tailp, on_act=True)
                    proj_block(t, 1, pool=tailp, on_act=True)

    nc.compile()
    return nc


def _emit_av(nc, avs, v_all, pair, prev, njt):
    # [attn-out^T ; denominators] accumulated over k-tiles; ones columns in
    # v_all put the denominators in output row 64.
    j, c0, pt = prev
    for h in range(2):
        hl = 2 * pair + h
        nc.tensor.matmul(
            avs[h][0:65, c0:512],
            v_all[:, j * 390 + hl * 65:j * 390 + hl * 65 + 65],
            pt[:, h * 512 + c0:(h + 1) * 512],
            start=(j == 0), stop=(j == njt - 1),
        )


def _numpy_fallback(x, mask, W_attn, b_attn, W_proj, b_proj):
    qkv = x @ W_attn + b_attn
    q, k, v = np.split(qkv, 3, axis=-1)

    def heads(t):
        return t.reshape(B, S, N_HEAD, HEAD_DIM).transpose(0, 2, 1, 3)

    q, k, v = heads(q), heads(k), heads(v)
    attn = np.einsum("bhqd,bhkd->bhqk", q, k) / np.sqrt(np.float32(HEAD_DIM))
    attn = attn + mask * (-1e9)
    attn = attn - attn.max(axis=-1, keepdims=True)
    attn = np.exp(attn)
    attn = attn / attn.sum(axis=-1, keepdims=True)
    out = np.einsum("bhqk,bhkd->bhqd", attn, v)
    out = out.transpose(0, 2, 1, 3).reshape(B, S, N_EMBD)
    return (out @ W_proj + b_proj).astype(np.float32)


def kernel(x, mask, W_attn, b_attn, W_proj, b_proj):
    global LAST_RESULTS, _PROGRAM
    x = np.asarray(x, dtype=np.float32)
    mask = np.asarray(mask, dtype=np.float32)
    W_attn = np.asarray(W_attn, dtype=np.float32)
    b_attn = np.asarray(b_attn, dtype=np.float32)
    W_proj = np.asarray(W_proj, dtype=np.float32)
    b_proj = np.asarray(b_proj, dtype=np.float32)

    # the kernel exploits causal structure; verify the mask actually is causal
    causal = 1.0 - np.tril(np.ones((S, S), dtype=np.float32))
    if mask.shape != (1, 1, S, S) or not np.array_equal(mask[0, 0], causal):
        return _numpy_fallback(x, mask, W_attn, b_attn, W_proj, b_proj)

    from concourse.bass_utils import run_bass_kernel_spmd

    if _PROGRAM is None:
        _PROGRAM = _build_program()

    in_maps = make_in_maps(x, W_attn, b_attn, W_proj)

    trace = bool(int(os.environ.get("ATTN_KERNEL_TRACE", "0")))
    res = run_bass_kernel_spmd(_PROGRAM, in_maps, list(range(N_CORES)), trace=trace)
    LAST_RESULTS = res

    y = np.zeros((B, S, N_EMBD), dtype=np.float32)
    for c in range(N_CORES):
        y[c // 2] += res.results[c]["y"]
    # softmax rows sum to 1: v-bias contributes b_v @ W_proj, a constant
    y += b_attn[1536:] @ W_proj + b_proj
    return y


def _pack_w_blocks(Wq, Wk, Wv):
    """[q0 | k0 | v(all) | q1 | k1 | q2 | k2], each block k-major [128, 6*w]."""
    def blk(W, m0, m1):
        return np.concatenate(
            [W[k * 128:(k + 1) * 128, m0 * 128:m1 * 128] for k in range(6)],
            axis=1)
    return np.ascontiguousarray(np.concatenate(
        [blk(Wq, 0, 1), blk(Wk, 0, 1), blk(Wv, 0, 3), blk(Wq, 1, 2),
         blk(Wk, 1, 2), blk(Wq, 2, 3), blk(Wk, 2, 3)], axis=1))


def make_in_maps(x, W_attn, b_attn, W_proj):
    bf16 = ml_dtypes.bfloat16
    in_maps = []
    for c in range(N_CORES):
        b, hg = divmod(c, 2)
        o = HG_DIM * hg
        in_maps.append({
            "xt": np.ascontiguousarray(x[b].astype(bf16).T),
            "w_blk": _pack_w_blocks(
                W_attn[:, o:o + HG_DIM].astype(bf16),
                W_attn[:, 768 + o:768 + o + HG_DIM].astype(bf16),
                W_attn[:, 1536 + o:1536 + o + HG_DIM].astype(bf16)),
            "b_qk": np.ascontiguousarray(np.concatenate(
                [b_attn[o:o + HG_DIM], b_attn[768 + o:768 + o + HG_DIM]])),
            "w_proj": np.ascontiguousarray(W_proj[o:o + HG_DIM, :].astype(bf16)),
            "ones": np.ones((1, 128), dtype=np.float32),
        })
    return in_maps


# revision 48
# speedup vs baseline: 1.1996x; 1.0042x over previous
"""Trainium2 Bass kernel for a 12-head causal attention block (GPT-2 style).

Problem: x:[4,2048,768] -> qkv = x@W_attn+b_attn, causal softmax attention
(12 heads, d=64), out @ W_proj + b_proj.

Sharding over 8 NeuronCores: core c handles batch b=c//2 (data parallel) and
head-group hg=c%2 (6 heads, tensor parallel on the qkv columns / proj rows).
Each core returns a partial projection output; the host sums the two
head-group partials per batch and adds the output bias (b_proj plus the
b_v@W_proj term: softmax rows sum to 1, so the v-bias contributes a constant
vector to the attention output and is folded host-side).

Per-core dataflow (inputs bf16; matmul accumulation fp32):
  - xT [emb, seq] comes straight from DRAM via DMA-transpose (bf16).
  - qkT = W-tiles.T @ xT -> qT,kT per head-pair [128,2048] (even head rows
    0-63, odd head rows 64-127); v in natural [seq, d] layout interleaved
    with ones columns (ones give the softmax denominators for free in the
    P@V matmul's 65th output row).
  - scores S^T[k,q] per 128k x 512q block: the two heads of a pair run
    ROW-PACKED (tile_position (0,0)/(64,0)) and execute concurrently in the
    PE array; one ACT exp call covers both heads.  Upper-triangle blocks are
    skipped; diagonal-crossing triangles zeroed post-exp with one 3D-batched
    gpsimd affine_select.  No max-subtraction needed (|scores/8| small).
  - P@V accumulates [attn-out^T ; den] in PSUM over k-tiles (M=65).
  - normalization: DVE reciprocal_approx_fast on the den row (~18 bits,
    5x faster than exact), gpsimd partition_broadcast to 64 rows, DVE
    multiply -> attnT (bf16).  Odd head bounced to rows 64-127 via DMA.
  - proj: y[128q,768] accumulated over the 3 head-pair k-tiles in 384-col
    chunks.
  - PE saturation: the attention j-loop is ACT(exp)-bound, so the qkv
    projection matmuls for LATER pairs and the output projection are emitted
    as filler blocks interleaved between j-iterations, keeping the PE busy
    (and HAM-warm) throughout instead of running phases serially.
"""

import os
import ml_dtypes
import numpy as np

N_HEAD = 12
N_EMBD = 768
HEAD_DIM = 64
B, S = 4, 2048
N_CORES = 8
HG_HEADS = 6            # heads per core (3 pairs)
HG_DIM = HG_HEADS * HEAD_DIM   # 384
QKV_W = 3 * HG_DIM      # 1152 qkv columns per core
N_PAIRS = 3
ST = S // 128           # 16 seq tiles of 128
NG = S // 512           # 4 seq groups of 512

# last run's BassKernelResults (test.py reads this for HW timing / traces)
LAST_RESULTS = None
_PROGRAM = None


def _build_program():
    """Build (once) the SPMD Bass program run identically on all 8 cores."""
    import concourse.bacc as bacc
    import concourse.tile as tile
    from concourse import mybir

    F32R = mybir.dt.float32r
    F32 = mybir.dt.float32
    BF16 = mybir.dt.bfloat16
    AF = mybir.ActivationFunctionType

    nc = bacc.Bacc(None, target_bir_lowering=False)
    xt_d = nc.declare_dram_parameter("xt", [N_EMBD, S], BF16, isOutput=False)
    # host-packed weight blocks, contiguous in consumption order:
    # [q0 | k0 | v(all pairs) | q1 | k1 | q2 | k2], each block k-major
    wblk_d = nc.declare_dram_parameter("w_blk", [128, 6 * QKV_W], BF16, isOutput=False)
    bqk_d = nc.declare_dram_parameter("b_qk", [768], F32, isOutput=False)
    ones_d = nc.declare_dram_parameter("ones", [1, 128], F32R, isOutput=False)
    wproj_d = nc.declare_dram_parameter("w_proj", [HG_DIM, N_EMBD], BF16, isOutput=False)
    y_d = nc.declare_dram_parameter("y", [S, N_EMBD], F32, isOutput=True)

    with tile.TileContext(nc) as tc:
        from contextlib import ExitStack

        with ExitStack() as outer:
            consts = outer.enter_context(tc.tile_pool(name="consts", bufs=1))
            ones_row = consts.tile([1, 128], F32R)
            nc.sync.dma_start(out=ones_row[:], in_=ones_d[:])
            bias_qk = consts.tile([128, 6], F32)      # col m: b_qk[128m:128m+128]
            nc.sync.dma_start(
                out=bias_qk[:], in_=bqk_d[0:768].rearrange("(m p) -> p m", p=128)
            )

            # ---- persistent activations/weights in SBUF ----
            big = outer.enter_context(tc.tile_pool(name="big", bufs=1))
            xT = big.tile([128, 6 * S], BF16)      # [emb-part, k-tile*2048+seq]
            qkT = big.tile([128, 6 * S], BF16)     # m=0..2 qT pairs, m=3..5 kT pairs
            # per k-tile: 6 heads x (64 v-cols + a ones col for the softmax
            # denominator) -> P@V and row-sums come from one M=65 matmul
            v_all = big.tile([128, ST * 390], BF16)  # [seq, t*390 + 65h + d]
            nc.gpsimd.memset(v_all[:], 1.0)
            attnT = big.tile([128, N_PAIRS * S], BF16)  # [pair d, pair*2048+seq]
            w_all = big.tile([128, 6 * QKV_W], BF16)
            w_proj = big.tile([128, N_PAIRS * N_EMBD], BF16)

            # preload the exp table set while DMAs stream in
            dummy = consts.tile([1, 128], F32)
            nc.scalar.activation(dummy[:], ones_row[:].bitcast(F32), AF.Exp,
                                 bias=0.0, scale=0.125)
            # seeded garbage tile for PE warm-keeper matmuls
            garbage = consts.tile([128, 512], BF16)
            nc.sync.dma_start(out=garbage[0:1, 0:128],
                              in_=ones_d[:].bitcast(BF16)[:, 0:128])

            # input DMAs, in consumption order: pair-0 q/k weights, the g0
            # columns of xT (unblocks the first qk block), v weights, the
            # rest of xT, remaining q/k weights.  All DRAM reads contiguous:
            # x pre-transposed and weights pre-packed by the host.
            xT_v = xT[:].rearrange("p (k s) -> p k s", k=6)
            w_view = w_all[:].rearrange("p (k c) -> p k c", k=6)

            def dma_w(blk_off, width, c0):
                nc.sync.dma_start(
                    out=w_view[:, :, c0:c0 + width],
                    in_=wblk_d[:, blk_off:blk_off + 6 * width].rearrange(
                        "p (k c) -> p k c", k=6),
                )

            def dma_x(gq):
                for k in range(6):
                    nc.sync.dma_start(
                        out=xT_v[:, k, gq * 512:(gq + 1) * 512],
                        in_=xt_d[k * 128:(k + 1) * 128, gq * 512:(gq + 1) * 512],
                    )

            dma_w(0, 128, 0)        # q pair 0
            dma_w(768, 128, 384)    # k pair 0
            dma_x(0)
            dma_w(1536, 384, 768)   # v columns, all pairs
            dma_x(1)
            dma_x(2)
            dma_x(3)
            dma_w(3840, 128, 128)   # q pair 1
            dma_w(4608, 128, 512)   # k pair 1
            dma_w(5376, 128, 256)   # q pair 2
            dma_w(6144, 128, 640)   # k pair 2
            for p in range(N_PAIRS):
                nc.sync.dma_start(
                    out=w_proj[:, p * N_EMBD:(p + 1) * N_EMBD],
                    in_=wproj_d[p * 128:(p + 1) * 128, :],
                )

            # ---- filler blocks: qkv projections + output projection ----
            fill = outer.enter_context(
                tc.tile_pool(name="fill", bufs=1, space="PSUM"))
            ys_pool = outer.enter_context(tc.tile_pool(name="ys", bufs=2))
            v_v = v_all[:].rearrange("p (t h d) -> p t h d", t=ST, h=6)

            def qk_block(m, g):
                # qT (m=pair) / kT (m=3+pair) for one 512-col seq group
                ps = fill.tile([128, 512], F32, tag="fill")
                for k in range(6):
                    nc.tensor.matmul(
                        ps[:],
                        w_all[:, k * QKV_W + m * 128:k * QKV_W + (m + 1) * 128],
                        xT_v[:, k, g * 512:(g + 1) * 512],
                        start=(k == 0), stop=(k == 5),
                    )
                nc.vector.tensor_scalar_add(
                    qkT[:, m * S + g * 512:m * S + (g + 1) * 512],
                    ps[:], bias_qk[:, m:m + 1],
                )

            def v_block(pr, t, npr=1):
                # v (npr head pairs from pr) for one 128-row seq tile
                ps = fill.tile([128, 512], F32, tag="fill")
                vc = 768 + pr * 128
                w = npr * 128
                for k in range(6):
                    nc.tensor.matmul(
                        ps[:, 0:w],
                        xT_v[:, k, t * 128:(t + 1) * 128],
                        w_all[:, k * QKV_W + vc:k * QKV_W + vc + w],
                        start=(k == 0), stop=(k == 5),
                    )
                nc.vector.tensor_copy(
                    v_v[:, t, 2 * pr:2 * pr + 2 * npr, 0:64],
                    ps[:, 0:w].rearrange("p (h d) -> p h d", h=2 * npr),
                )

            def proj_block(t, half, pool=None, on_act=False):
                # y[:, 384*half : 384*(half+1)] for one 128-row seq tile
                ps = (pool or fill).tile([128, 384], F32, tag="fill")
                h0 = 384 * half
                for p in range(N_PAIRS):
                    nc.tensor.matmul(
                        ps[:],
                        attnT[:, p * S + t * 128:p * S + (t + 1) * 128],
                        w_proj[:, p * N_EMBD + h0:p * N_EMBD + h0 + 384],
                        start=(p == 0), stop=(p == N_PAIRS - 1),
                    )
                ys = ys_pool.tile([128, 384], F32)
                if on_act:   # tail: ACT is idle, DVE is the serializer
                    nc.scalar.copy(ys[:], ps[:])
                else:
                    nc.vector.tensor_copy(ys[:], ps[:])
                nc.sync.dma_start(
                    out=y_d[t * 128:(t + 1) * 128, h0:h0 + 384], in_=ys[:])

            # filler emission schedule: blocks spread across the j-loops of
            # each (pair, g) attention group, ordered so every block lands
            # before its consumer group starts.
            spread = {
                (0, 0): [(v_block, 0, 0, 2), (v_block, 0, 1, 2),
                         (v_block, 0, 2, 2), (v_block, 0, 3, 2),
                         (qk_block, 0, 1), (qk_block, 3, 1)],
                (0, 1): [(v_block, 0, 4, 2), (v_block, 0, 5, 2),
                         (v_block, 0, 6, 2), (v_block, 0, 7, 2),
                         (qk_block, 0, 2), (qk_block, 3, 2)],
                (0, 2): [(v_block, 0, 8, 2), (v_block, 0, 9, 2),
                         (v_block, 0, 10, 2), (v_block, 0, 11, 2),
                         (qk_block, 0, 3), (qk_block, 3, 3),
                         (qk_block, 1, 0), (qk_block, 4, 0)],
                (0, 3): [(v_block, 0, 12, 2), (v_block, 0, 13, 2),
                         (v_block, 0, 14, 2), (v_block, 0, 15, 2),
                         (qk_block, 1, 1), (qk_block, 4, 1)],
                (1, 0): [(qk_block, 1, 2), (qk_block, 4, 2),
                         (v_block, 2, 0), (v_block, 2, 1)],
                (1, 1): [(qk_block, 1, 3), (qk_block, 4, 3),
                         (v_block, 2, 2), (v_block, 2, 3),
                         (v_block, 2, 4), (v_block, 2, 5),
                         (qk_block, 2, 0), (qk_block, 5, 0)],
                (1, 2): [(v_block, 2, 6), (v_block, 2, 7),
                         (v_block, 2, 8), (v_block, 2, 9),
                         (qk_block, 2, 1), (qk_block, 5, 1)],
                (1, 3): [(v_block, 2, 10), (v_block, 2, 11),
                         (v_block, 2, 12), (v_block, 2, 13),
                         (v_block, 2, 14), (v_block, 2, 15),
                         (qk_block, 2, 2), (qk_block, 5, 2)],
                (2, 0): [(qk_block, 2, 3), (qk_block, 5, 3)],
                (2, 1): [(proj_block, t, h) for t in range(4) for h in (0, 1)],
                (2, 2): [(proj_block, t, h) for t in range(4, 8) for h in (0, 1)],
                (2, 3): [(proj_block, t, h) for t in range(8, 12) for h in (0, 1)],
            }

            # ---- head: first pair's g=0 q/k (v blocks are (0,0) fillers).
            # Dummy matmuls on the early-seeded garbage tile warm the PE HAM
            # clock gate (1.2 -> 2.4 GHz) during the input-DMA wait.
            warmh = fill.tile([128, 512], F32, tag="fill")
            for i in range(8):
                nc.tensor.matmul(warmh[:], garbage[:, 0:128], garbage[:],
                                 start=True, stop=True)
            # both pair-0 qk blocks interleaved on two PSUM slots, so each
            # xT chunk arrival feeds two matmuls and the blocks don't
            # serialize on a single bank + bias-add evacuation
            with tc.tile_pool(name="hps", bufs=2, space="PSUM") as hps:
                psq = hps.tile([128, 512], F32, tag="h")
                psk = hps.tile([128, 512], F32, tag="h")
                for k in range(6):
                    nc.tensor.matmul(
                        psq[:], w_all[:, k * QKV_W:k * QKV_W + 128],
                        xT_v[:, k, 0:512], start=(k == 0), stop=(k == 5))
                    nc.tensor.matmul(
                        psk[:], w_all[:, k * QKV_W + 384:k * QKV_W + 512],
                        xT_v[:, k, 0:512], start=(k == 0), stop=(k == 5))
                nc.vector.tensor_scalar_add(
                    qkT[:, 0:512], psq[:], bias_qk[:, 0:1])
                nc.vector.tensor_scalar_add(
                    qkT[:, 3 * S:3 * S + 512], psk[:], bias_qk[:, 3:4])

            # ---- attention: ACT-bound j-loops with PE filler interleave ----
            with tc.tile_pool(name="stps", bufs=2, space="PSUM") as stps, \
                 tc.tile_pool(name="avps", bufs=3, space="PSUM") as avps, \
                 tc.tile_pool(name="ptp", bufs=5) as ptp, \
                 tc.tile_pool(name="rcp", bufs=2) as rcp, \
                 tc.tile_pool(name="bcp", bufs=2) as bcp, \
                 tc.tile_pool(name="avcp", bufs=2) as avcp, \
                 tc.tile_pool(name="shtmp", bufs=2) as shtmp:
                for pair in range(N_PAIRS):
                    q0 = pair * S          # qT pair tile offset in qkT
                    k0 = (3 + pair) * S    # kT pair tile offset
                    for g in range(NG):
                        av0 = avps.tile([65, 512], F32, tag="av")
                        av1 = avps.tile([65, 512], F32, tag="av")
                        avs = (av0, av1)
                        njt = 4 * g + 4
                        fills = list(spread[(pair, g)])
                        nfill = len(fills)
                        avq = []  # software-pipeline AV two j behind
                        for j in range(njt):
                            diag_r = j - 4 * g   # >=0 on diagonal tiles
                            c0 = 128 * diag_r if diag_r >= 0 else 0
                            st = stps.tile([128, 1024], F32, tag="st")
                            pt = ptp.tile([128, 1024], BF16, tag="pt")
                            # row-packed scores: both heads concurrently
                            nc.tensor.matmul(
                                st[:, c0:512],
                                qkT[0:64, k0 + j * 128:k0 + (j + 1) * 128],
                                qkT[0:64, q0 + g * 512 + c0:q0 + (g + 1) * 512],
                                start=True, stop=True, tile_position=(0, 0),
                            )
                            nc.tensor.matmul(
                                st[:, 512 + c0:1024],
                                qkT[64:128, k0 + j * 128:k0 + (j + 1) * 128],
                                qkT[64:128, q0 + g * 512 + c0:q0 + (g + 1) * 512],
                                start=True, stop=True, tile_position=(64, 0),
                            )
                            # exp(S/8) over both heads' valid columns
                            nc.scalar.activation(
                                pt[:, c0:1024], st[:, c0:1024], AF.Exp,
                                bias=0.0, scale=0.125,
                            )
                            if diag_r >= 0:
                                # zero the strictly-lower (k>q) triangle of
                                # both heads in one 3D-batched op
                                p3 = pt[:].rearrange("p (h s) -> p h s", h=2)
                                nc.gpsimd.affine_select(
                                    out=p3[:, :, c0:c0 + 128],
                                    in_=p3[:, :, c0:c0 + 128],
                                    compare_op=mybir.AluOpType.is_ge,
                                    fill=0.0, base=0,
                                    pattern=[[0, 2], [1, 128]],
                                    channel_multiplier=-1,
                                )
                            avq.append((j, c0, pt))
                            if len(avq) > 3:
                                _emit_av(nc, avs, v_all, pair, avq.pop(0), njt)
                            # PE filler between j iterations
                            while fills and len(fills) > (nfill * (njt - 1 - j)) // njt:
                                blk = fills.pop(0)
                                blk[0](*blk[1:])
                        for prev in avq:
                            _emit_av(nc, avs, v_all, pair, prev, njt)

                        # ---- normalization tail ----
                        cols = slice(pair * S + g * 512, pair * S + (g + 1) * 512)
                        for h in range(2):
                            # den row 64 -> partition 0 (plain copies handle
                            # the shift; reciprocal_approx_fast does NOT work
                            # on base-partition-64 APs)
                            rc = rcp.tile([1, 512], F32, tag="rc")
                            nc.vector.tensor_copy(rc[:], avs[h][64:65, :])
                            nc.vector.reciprocal_approx_fast(rc[:], rc[:])
                            bc = bcp.tile([64, 512], F32, tag="bc")
                            nc.gpsimd.partition_broadcast(bc[:], rc[:])
                            if h == 0:
                                nc.vector.tensor_mul(
                                    attnT[0:64, cols], avs[h][0:64, :], bc[:])
                            else:
                                # DVE lanes are partition-locked: odd head's
                                # rows 64-127 go via an SBUF bounce + DMA
                                tmp = shtmp.tile([64, 512], BF16, tag="sh")
                                nc.vector.tensor_mul(
                                    tmp[:], avs[h][0:64, :], bc[:])
                                nc.sync.dma_start(out=attnT[64:128, cols],
                                                  in_=tmp[:])
                        if pair == 2 and g == 3:
                            # bridge the PE idle of the final normalization
                            # chain so the tail projection runs at 2.4 GHz
                            wps = fill.tile([128, 512], F32, tag="fill")
                            for i in range(12):
                                nc.tensor.matmul(
                                    wps[:, 0:384], garbage[0:64, 0:128],
                                    tmp[:, 0:384], start=True, stop=True)

            # ---- remaining output projection (PSUM free: deep-buffer it).
            # Dummy matmuls bridge the PE idle window during the last
            # normalization so the HAM clock stays at 2.4 GHz for the tail.
            with tc.tile_pool(name="tailp", bufs=3, space="PSUM") as tailp:
                warm = tailp.tile([128, 384], F32, tag="warm")
                for i in range(14):
                    nc.tensor.matmul(warm[:], garbage[:, 0:128],
                                     garbage[:, 0:384], start=True, stop=True)
                for t in range(12, ST):
                    proj_block(t, 0, pool=tailp, on_act=True)
                    proj_block(t, 1, pool=tailp, on_act=True)

    nc.compile()
    return nc


def _emit_av(nc, avs, v_all, pair, prev, njt):
    # [attn-out^T ; denominators] accumulated over k-tiles; ones columns in
    # v_all put the denominators in output row 64.
    j, c0, pt = prev
    for h in range(2):
        hl = 2 * pair + h
        nc.tensor.matmul(
            avs[h][0:65, c0:512],
            v_all[:, j * 390 + hl * 65:j * 390 + hl * 65 + 65],
            pt[:, h * 512 + c0:(h + 1) * 512],
            start=(j == 0), stop=(j == njt - 1),
        )


def _numpy_fallback(x, mask, W_attn, b_attn, W_proj, b_proj):
    qkv = x @ W_attn + b_attn
    q, k, v = np.split(qkv, 3, axis=-1)

    def heads(t):
        return t.reshape(B, S, N_HEAD, HEAD_DIM).transpose(0, 2, 1, 3)

    q, k, v = heads(q), heads(k), heads(v)
    attn = np.einsum("bhqd,bhkd->bhqk", q, k) / np.sqrt(np.float32(HEAD_DIM))
    attn = attn + mask * (-1e9)
    attn = attn - attn.max(axis=-1, keepdims=True)
    attn = np.exp(attn)
    attn = attn / attn.sum(axis=-1, keepdims=True)
    out = np.einsum("bhqk,bhkd->bhqd", attn, v)
    out = out.transpose(0, 2, 1, 3).reshape(B, S, N_EMBD)
    return (out @ W_proj + b_proj).astype(np.float32)


def kernel(x, mask, W_attn, b_attn, W_proj, b_proj):
    global LAST_RESULTS, _PROGRAM
    x = np.asarray(x, dtype=np.float32)
    mask = np.asarray(mask, dtype=np.float32)
    W_attn = np.asarray(W_attn, dtype=np.float32)
    b_attn = np.asarray(b_attn, dtype=np.float32)
    W_proj = np.asarray(W_proj, dtype=np.float32)
    b_proj = np.asarray(b_proj, dtype=np.float32)

    # the kernel exploits causal structure; verify the mask actually is causal
    causal = 1.0 - np.tril(np.ones((S, S), dtype=np.float32))
    if mask.shape != (1, 1, S, S) or not np.array_equal(mask[0, 0], causal):
        return _numpy_fallback(x, mask, W_attn, b_attn, W_proj, b_proj)

    from concourse.bass_utils import run_bass_kernel_spmd

    if _PROGRAM is None:
        _PROGRAM = _build_program()

    in_maps = make_in_maps(x, W_attn, b_attn, W_proj)

    trace = bool(int(os.environ.get("ATTN_KERNEL_TRACE", "0")))
    res = run_bass_kernel_spmd(_PROGRAM, in_maps, list(range(N_CORES)), trace=trace)
    LAST_RESULTS = res

    y = np.zeros((B, S, N_EMBD), dtype=np.float32)
    for c in range(N_CORES):
        y[c // 2] += res.results[c]["y"]
    # softmax rows sum to 1: v-bias contributes b_v @ W_proj, a constant
    y += b_attn[1536:] @ W_proj + b_proj
    return y


def _pack_w_blocks(Wq, Wk, Wv):
    """[q0 | k0 | v(all) | q1 | k1 | q2 | k2], each block k-major [128, 6*w]."""
    def blk(W, m0, m1):
        return np.concatenate(
            [W[k * 128:(k + 1) * 128, m0 * 128:m1 * 128] for k in range(6)],
            axis=1)
    return np.ascontiguousarray(np.concatenate(
        [blk(Wq, 0, 1), blk(Wk, 0, 1), blk(Wv, 0, 3), blk(Wq, 1, 2),
         blk(Wk, 1, 2), blk(Wq, 2, 3), blk(Wk, 2, 3)], axis=1))


def make_in_maps(x, W_attn, b_attn, W_proj):
    bf16 = ml_dtypes.bfloat16
    in_maps = []
    for c in range(N_CORES):
        b, hg = divmod(c, 2)
        o = HG_DIM * hg
        in_maps.append({
            "xt": np.ascontiguousarray(x[b].astype(bf16).T),
            "w_blk": _pack_w_blocks(
                W_attn[:, o:o + HG_DIM].astype(bf16),
                W_attn[:, 768 + o:768 + o + HG_DIM].astype(bf16),
                W_attn[:, 1536 + o:1536 + o + HG_DIM].astype(bf16)),
            "b_qk": np.ascontiguousarray(np.concatenate(
                [b_attn[o:o + HG_DIM], b_attn[768 + o:768 + o + HG_DIM]])),
            "w_proj": np.ascontiguousarray(W_proj[o:o + HG_DIM, :].astype(bf16)),
            "ones": np.ones((1, 128), dtype=np.float32),
        })
    return in_maps
